# revision 40
# baseline (speedup 1.0000x reference)
"""DEMA (double exponential smoothing) Trainium2 Bass kernel.

Math
----
Reference recurrence (per batch b, channel c, over time t):
    s0 = x[0], b0 = x[1] - x[0]
    s_t = a*x_t + (1-a)*(s_{t-1} + b_{t-1})
    b_t = bt*(s_t - s_{t-1}) + (1-bt)*b_{t-1}
    out = [s0, s_1, ..., s_{T-1}]

Eliminating the trend state gives a linear constant-coefficient 2nd-order
recurrence (exact; s_0 = x_0, s_1 = x_1):
    s_t = tau*s_{t-1} - delta*s_{t-2} + b0*x_t + b1*x_{t-1},  t >= 2
    tau = 2 - a - a*bt, delta = 1 - a, b0 = a, b1 = a*((1-a)*(1+bt) - tau)

So out = M @ x along time, where M is lower-triangular with Toeplitz body
M[t,k] = w_{t-k} (w = impulse response, w_j = tau*w_{j-1} - delta*w_{j-2})
plus two special leading columns for the x_0/x_1 initial conditions. The
poles satisfy |lambda| <= sqrt(1-a) < 1, so w decays geometrically and M
is effectively banded: blocking time into 128-chunks, out-block i only
needs input blocks j >= i-D, where D is chosen on host so the dropped
tail is below 1e-8 relative (D=1 for the graded PRNG alpha/beta).

The kernel is a causal blocked convolution on the TensorEngine:
    out_blk[i] = sum_{d=0..min(i,D)} W_d^T @ x_blk[i-d]       (PSUM accum)
with 128x128 weight blocks W_d (plus special j=0 variants carrying the
initial-condition columns) computed on host in float64 from the runtime
alpha/beta and shipped as a small input tensor.

Performance notes (236.6us fp32 ancestor -> 96.4-110us measured;
run-to-run spread is HBM/DMA service-rate noise across runs — judge
changes by trace structure, not single wall-clock samples):
- Everything moves in float16 (x, weights, y; PSUM accumulates fp32).
  The graded accuracy gate is 2e-2 relative; fp16 quantization costs
  ~3.6e-4. PE time drops 4x (fp16 is 1 pass/column vs 4 for fp32) and
  HBM traffic halves to ~33.7 MB/core.
- Host ships x batch-transposed as [128, NBLK*C]: partition t holds
  row t of every 128-block back to back, so a whole-batch DMA is 128
  packets of NBLK*C*2 = 32 KiB contiguous HBM each. The fp32 ancestor
  moved one 2 KiB row per packet. The 16 DMA engines cap at ~27 GB/s
  each (~430 GB/s/core aggregate, reached for packets >= 8 KiB), so
  the 33.7 MB floor is ~80us of saturated DMA: this kernel is
  DMA-bound, with loads and stores each needing ~215 GB/s sustained.
  y uses the same layout; the host inverse transpose/cast is not on
  the HW critical path.
- Queue topology (measured, counterintuitive): ALL loads on the sync
  ring in exact consumption order (a second load ring lets later
  batches' descriptors race ahead of the chunk the PE needs next —
  priority inversion), stores on the gpsimd ring (the only
  DMA-capable engine with no other work: store issues on
  scalar/vector stall evictions because engines execute in order),
  PSUM evictions alternate ScalarE/VectorE. Loads go as half-batch
  (16-block) sub-DMAs: 16 KiB packets hit the same ~27 GB/s/engine as
  32 KiB, and halving the batch-boundary gate cut total PE stall from
  ~21us to ~13us (the PE outruns its ~50% share of the load stream,
  so it waits at every batch start). Stores go out in 8-block chunks
  so the store stream starts ~14us in and the DMA engines serve both
  directions wall-to-wall.
- Batch 0 ramps with small load/store chunks (first matmul at ~11us)
  and the last batch tapers its stores (8,8,8,4,2,2) so the final
  load->matmul->evict->store chain stays short.

Sharding: batch 32 -> 4 per core across 8 cores (data parallel; the
recurrence is independent per (b, c)).
"""

import numpy as np

import concourse.bacc as bacc
import concourse.bass as bass
import concourse.mybir as mybir
from concourse import tile
from concourse.bass_utils import run_bass_kernel_spmd

N_CORES = 8
P = 128            # SBUF partitions == time-block length
B, T, C = 32, 4096, 512
BC = B // N_CORES  # batches per core
NBLK = T // P      # 32 time blocks
CHUNK = 8          # blocks per fill/drain chunk (8*512*2B = 8 KiB/partition)

_F32 = mybir.dt.float32
_MM_DT = mybir.dt.float16
_NP_DT = np.float16


def _host_weights(a: float, bt: float, tol: float = 1e-8):
    """Impulse response + IC columns -> (D, wts[2*(D+1), 128, 128]) lhsT-layout."""
    tau = 2.0 - a - a * bt
    delta = 1.0 - a
    b0 = a
    b1 = a * ((1.0 - a) * (1.0 + bt) - tau)
    n = T
    w = np.zeros(n)
    c0 = np.zeros(n)
    c1 = np.zeros(n)
    w[0] = b0
    w[1] = tau * b0 + b1
    c0[0] = 1.0
    c1[1] = 1.0
    for j in range(2, n):
        w[j] = tau * w[j - 1] - delta * w[j - 2]
        c0[j] = tau * c0[j - 1] - delta * c0[j - 2]
        c1[j] = tau * c1[j - 1] - delta * c1[j - 2] + (b1 if j == 2 else 0.0)
    wnorm = max(np.sqrt((w ** 2).sum()), 1.0)
    D = NBLK - 1
    for d in range(NBLK):
        tail = np.sqrt(
            (w[P * d + 1 :] ** 2).sum()
            + (c0[P * (d + 1) :] ** 2).sum()
            + (c1[P * (d + 1) :] ** 2).sum()
        )
        if tail <= tol * wnorm:
            D = d
            break
    # lhsT layout [k, t]: out[t, n] = sum_k W[k, t] * x[k, n]
    wts = np.zeros((2 * (D + 1), P, P), np.float32)
    kk = np.arange(P)[:, None]
    tt = np.arange(P)[None, :]
    for d in range(D + 1):
        lag = P * d + tt - kk          # [k, t] lag matrix
        Tm = np.where((lag >= 0) & (lag < n), w[np.clip(lag, 0, n - 1)], 0.0)
        Sm = Tm.copy()
        Sm[0, :] = c0[P * d : P * d + P]
        Sm[1, :] = c1[P * d : P * d + P]
        wts[2 * d] = Tm
        wts[2 * d + 1] = Sm
    return D, wts


def _build(D, bcount=BC, t_len=T, c_len=C):
    """Build + compile the per-core SPMD module for diagonal depth D."""
    nblk = t_len // P
    nw = 2 * (D + 1)
    nc = bacc.Bacc("TRN2", target_bir_lowering=False, debug=False)
    # x/y live in HBM batch-transposed: [b, t_in_block, blk*C + c]
    x = nc.dram_tensor("x", [bcount, P, nblk * c_len], _MM_DT, kind="ExternalInput")
    wd = nc.dram_tensor("wts", [P, nw * P], _MM_DT, kind="ExternalInput")
    y = nc.dram_tensor("y", [bcount, P, nblk * c_len], _MM_DT, kind="ExternalOutput")

    with tile.TileContext(nc) as tc:
        with (
            tc.tile_pool(name="wpool", bufs=1) as wpool,
            tc.tile_pool(name="xpool", bufs=3) as xpool,
            tc.tile_pool(name="psum", bufs=8, space="PSUM") as pspool,
            tc.tile_pool(name="opool", bufs=4) as opool,
        ):
            wt = wpool.tile([P, nw * P], _MM_DT)
            nc.sync.dma_start(wt[:], wd[:])

            for b in range(bcount):
                xb = xpool.tile([P, nblk * c_len], _MM_DT, tag="xb")
                # chunked loads, alternating between the sync and gpsimd
                # DMA queues (both issuing engines are otherwise idle, so
                # issue is never delayed): with stores on a third
                # (scalar) queue, the per-ring round-robin of the 16 DMA
                # engines gives the load stream a 2/3 share, so loads
                # (which gate the PE, which gates evictions/stores)
                # finish early and stores soak up the engines afterwards.
                # Batch 0 ramps with small chunks so the first matmul
                # starts ASAP.
                # all loads on ONE ring (sync) in consumption order: a
                # second load ring lets later batches' descriptors race
                # ahead of the chunk the PE needs next (ring round-robin
                # is priority inversion). Batch 0 ramps finely so the
                # first matmul starts ~8.5us; later batches load as one
                # DMA (32 KiB packets, fewest ring slots).
                lchunks = [2, 2, 4, 8, 8, 8] if b == 0 else [16, 16]
                g = 0
                for cw in lchunks:
                    nc.sync.dma_start(
                        xb[:, g * c_len : (g + cw) * c_len],
                        x[b, :, g * c_len : (g + cw) * c_len],
                    )
                    g += cw
                # store per chunk so the store stream starts ~one chunk
                # after the load stream instead of one batch after: the
                # 16 DMA engines then serve both queues wall-to-wall.
                # The last batch tapers so the final store's
                # load->matmul->evict->store tail is short.
                last_b = b == bcount - 1
                if b == 0:
                    # small first chunks: the store stream starts ~11us
                    schunks = [2, 2, 4, 8, 8, 8]
                elif last_b:
                    # taper: the final load->matmul->evict->store chain
                    # after the last load packet stays short
                    schunks = [8, 8, 8, 4, 2, 2]
                else:
                    schunks = [8, 8, 8, 8]
                g = 0
                for cw in schunks:
                    oc = opool.tile([P, 8 * c_len], _MM_DT, tag="oc")
                    for ii in range(cw):
                        i = g + ii
                        ps = pspool.tile([P, c_len], _F32, tag="ps")
                        dmax = min(i, D)
                        for nd, d in enumerate(range(dmax, -1, -1)):
                            j = i - d
                            wsl = 2 * d + (1 if j == 0 else 0)
                            nc.tensor.matmul(
                                ps[:],
                                wt[:, wsl * P : (wsl + 1) * P],
                                xb[:, j * c_len : (j + 1) * c_len],
                                start=(nd == 0),
                                stop=(nd == dmax),
                            )
                        dst = oc[:, ii * c_len : (ii + 1) * c_len]
                        if i % 2 == 0:
                            nc.scalar.copy(dst, ps[:])
                        else:
                            nc.vector.tensor_copy(dst, ps[:])
                    # gpsimd queue: the only DMA-capable engine with no
                    # other work, so store issue never blocks evictions
                    # (scalar/vector) or loads (sync)
                    nc.gpsimd.dma_start(
                        y[b, :, g * c_len : (g + cw) * c_len],
                        oc[:, : cw * c_len],
                    )
                    g += cw
    nc.compile()
    return nc


_MODULE_CACHE: dict = {}


def _get_module(D, **kw):
    key = (D, tuple(sorted(kw.items())))
    if key not in _MODULE_CACHE:
        _MODULE_CACHE[key] = _build(D, **kw)
    return _MODULE_CACHE[key]


def make_in_maps(x, alpha, beta, bcount=BC, n_cores=N_CORES):
    a = float(np.asarray(alpha).reshape(-1)[0])
    bt = float(np.asarray(beta).reshape(-1)[0])
    D, wts = _host_weights(a, bt)
    nw = 2 * (D + 1)
    # [nw, k, t] -> [k, nw*t] so the weight DMA is contiguous per partition
    wts16 = np.ascontiguousarray(
        wts.transpose(1, 0, 2).reshape(P, nw * P), dtype=_NP_DT
    )
    # [b, t, c] -> [b, t_in_block, blk, c]: partition t's whole batch is
    # one contiguous NBLK*C*2 = 32 KiB HBM chunk per DMA packet
    xt = x.reshape(B, NBLK, P, C).transpose(0, 2, 1, 3)
    in_maps = []
    for i in range(n_cores):
        xs = np.ascontiguousarray(
            xt[i * bcount : (i + 1) * bcount], dtype=_NP_DT
        ).reshape(bcount, P, NBLK * C)
        in_maps.append({"x": xs, "wts": wts16})
    return D, in_maps


def _run(x, alpha, beta, trace=False, **kw):
    x = np.asarray(x, dtype=np.float32)
    assert x.shape == (B, T, C), x.shape
    D, in_maps = make_in_maps(x, alpha, beta)
    nc = _get_module(D)
    res = run_bass_kernel_spmd(nc, in_maps, list(range(N_CORES)), trace=trace, **kw)
    yt = np.concatenate([res.results[i]["y"] for i in range(N_CORES)], axis=0)
    # invert the batch transpose: [b, t_in_block, blk, c] -> [b, t, c]
    out = (
        yt.reshape(B, P, NBLK, C)
        .transpose(0, 2, 1, 3)
        .reshape(B, T, C)
        .astype(np.float32)
    )
    return out, res


def kernel(x, alpha, beta):
    return _run(x, alpha, beta)[0]


# revision 41
# speedup vs baseline: 1.0780x; 1.0780x over previous
"""DEMA (double exponential smoothing) Trainium2 Bass kernel.

Math
----
Reference recurrence (per batch b, channel c, over time t):
    s0 = x[0], b0 = x[1] - x[0]
    s_t = a*x_t + (1-a)*(s_{t-1} + b_{t-1})
    b_t = bt*(s_t - s_{t-1}) + (1-bt)*b_{t-1}
    out = [s0, s_1, ..., s_{T-1}]

Eliminating the trend state gives a linear constant-coefficient 2nd-order
recurrence (exact; s_0 = x_0, s_1 = x_1):
    s_t = tau*s_{t-1} - delta*s_{t-2} + b0*x_t + b1*x_{t-1},  t >= 2
    tau = 2 - a - a*bt, delta = 1 - a, b0 = a, b1 = a*((1-a)*(1+bt) - tau)

So out = M @ x along time, where M is lower-triangular with Toeplitz body
M[t,k] = w_{t-k} (w = impulse response, w_j = tau*w_{j-1} - delta*w_{j-2})
plus two special leading columns for the x_0/x_1 initial conditions. The
poles satisfy |lambda| <= sqrt(1-a) < 1, so w decays geometrically and M
is effectively banded: blocking time into 128-chunks, out-block i only
needs input blocks j >= i-D, where D is chosen on host so the dropped
tail is below 1e-8 relative (D=1 for the graded PRNG alpha/beta).

The kernel is a causal blocked convolution on the TensorEngine:
    out_blk[i] = sum_{d=0..min(i,D)} W_d^T @ x_blk[i-d]       (PSUM accum)
with 128x128 weight blocks W_d (plus special j=0 variants carrying the
initial-condition columns) computed on host in float64 from the runtime
alpha/beta and shipped as a small input tensor.

Performance notes (236.6us fp32 ancestor -> 96.4-110us measured;
run-to-run spread is HBM/DMA service-rate noise across runs — judge
changes by trace structure, not single wall-clock samples):
- Everything moves in float16 (x, weights, y; PSUM accumulates fp32).
  The graded accuracy gate is 2e-2 relative; fp16 quantization costs
  ~3.6e-4. PE time drops 4x (fp16 is 1 pass/column vs 4 for fp32) and
  HBM traffic halves to ~33.7 MB/core.
- Host ships x batch-transposed as [128, NBLK*C]: partition t holds
  row t of every 128-block back to back, so a whole-batch DMA is 128
  packets of NBLK*C*2 = 32 KiB contiguous HBM each. The fp32 ancestor
  moved one 2 KiB row per packet. The 16 DMA engines cap at ~27 GB/s
  each (~430 GB/s/core aggregate, reached for packets >= 8 KiB), so
  the 33.7 MB floor is ~80us of saturated DMA: this kernel is
  DMA-bound, with loads and stores each needing ~215 GB/s sustained.
  y uses the same layout; the host inverse transpose/cast is not on
  the HW critical path.
- Queue topology (measured, counterintuitive): ALL loads on the sync
  ring in exact consumption order (a second load ring lets later
  batches' descriptors race ahead of the chunk the PE needs next —
  priority inversion), stores on the gpsimd ring (the only
  DMA-capable engine with no other work: store issues on
  scalar/vector stall evictions because engines execute in order),
  PSUM evictions alternate ScalarE/VectorE. Loads go as half-batch
  (16-block) sub-DMAs: 16 KiB packets hit the same ~27 GB/s/engine as
  32 KiB, and halving the batch-boundary gate cut total PE stall from
  ~21us to ~13us (the PE outruns its ~50% share of the load stream,
  so it waits at every batch start). Stores go out in 8-block chunks
  so the store stream starts ~14us in and the DMA engines serve both
  directions wall-to-wall.
- Batch 0 ramps with small load/store chunks (first matmul at ~11us)
  and the last batch tapers its stores (8,8,8,4,2,2) so the final
  load->matmul->evict->store chain stays short.

Sharding: batch 32 -> 4 per core across 8 cores (data parallel; the
recurrence is independent per (b, c)).
"""

import numpy as np

import concourse.bacc as bacc
import concourse.bass as bass
import concourse.mybir as mybir
from concourse import tile
from concourse.bass_utils import run_bass_kernel_spmd

N_CORES = 8
P = 128            # SBUF partitions == time-block length
B, T, C = 32, 4096, 512
BC = B // N_CORES  # batches per core
NBLK = T // P      # 32 time blocks
CHUNK = 8          # blocks per fill/drain chunk (8*512*2B = 8 KiB/partition)

_F32 = mybir.dt.float32
_MM_DT = mybir.dt.float16
_NP_DT = np.float16


def _host_weights(a: float, bt: float, tol: float = 1e-8):
    """Impulse response + IC columns -> (D, wts[2*(D+1), 128, 128]) lhsT-layout."""
    tau = 2.0 - a - a * bt
    delta = 1.0 - a
    b0 = a
    b1 = a * ((1.0 - a) * (1.0 + bt) - tau)
    n = T
    w = np.zeros(n)
    c0 = np.zeros(n)
    c1 = np.zeros(n)
    w[0] = b0
    w[1] = tau * b0 + b1
    c0[0] = 1.0
    c1[1] = 1.0
    for j in range(2, n):
        w[j] = tau * w[j - 1] - delta * w[j - 2]
        c0[j] = tau * c0[j - 1] - delta * c0[j - 2]
        c1[j] = tau * c1[j - 1] - delta * c1[j - 2] + (b1 if j == 2 else 0.0)
    wnorm = max(np.sqrt((w ** 2).sum()), 1.0)
    D = NBLK - 1
    for d in range(NBLK):
        tail = np.sqrt(
            (w[P * d + 1 :] ** 2).sum()
            + (c0[P * (d + 1) :] ** 2).sum()
            + (c1[P * (d + 1) :] ** 2).sum()
        )
        if tail <= tol * wnorm:
            D = d
            break
    # lhsT layout [k, t]: out[t, n] = sum_k W[k, t] * x[k, n]
    wts = np.zeros((2 * (D + 1), P, P), np.float32)
    kk = np.arange(P)[:, None]
    tt = np.arange(P)[None, :]
    for d in range(D + 1):
        lag = P * d + tt - kk          # [k, t] lag matrix
        Tm = np.where((lag >= 0) & (lag < n), w[np.clip(lag, 0, n - 1)], 0.0)
        Sm = Tm.copy()
        Sm[0, :] = c0[P * d : P * d + P]
        Sm[1, :] = c1[P * d : P * d + P]
        wts[2 * d] = Tm
        wts[2 * d + 1] = Sm
    return D, wts


def _build(D, bcount=BC, t_len=T, c_len=C):
    """Build + compile the per-core SPMD module for diagonal depth D."""
    nblk = t_len // P
    nw = 2 * (D + 1)
    nc = bacc.Bacc("TRN2", target_bir_lowering=False, debug=False)
    # x/y live in HBM batch-transposed: [b, t_in_block, blk*C + c]
    x = nc.dram_tensor("x", [bcount, P, nblk * c_len], _MM_DT, kind="ExternalInput")
    wd = nc.dram_tensor("wts", [P, nw * P], _MM_DT, kind="ExternalInput")
    y = nc.dram_tensor("y", [bcount, P, nblk * c_len], _MM_DT, kind="ExternalOutput")

    with tile.TileContext(nc) as tc:
        with (
            tc.tile_pool(name="wpool", bufs=1) as wpool,
            tc.tile_pool(name="xpool", bufs=4) as xpool,
            tc.tile_pool(name="psum", bufs=8, space="PSUM") as pspool,
            tc.tile_pool(name="opool", bufs=4) as opool,
        ):
            wt = wpool.tile([P, nw * P], _MM_DT)
            nc.sync.dma_start(wt[:], wd[:])

            for b in range(bcount):
                xb = xpool.tile([P, nblk * c_len], _MM_DT, tag="xb")
                # chunked loads, alternating between the sync and gpsimd
                # DMA queues (both issuing engines are otherwise idle, so
                # issue is never delayed): with stores on a third
                # (scalar) queue, the per-ring round-robin of the 16 DMA
                # engines gives the load stream a 2/3 share, so loads
                # (which gate the PE, which gates evictions/stores)
                # finish early and stores soak up the engines afterwards.
                # Batch 0 ramps with small chunks so the first matmul
                # starts ASAP.
                # all loads on ONE ring (sync) in consumption order: a
                # second load ring lets later batches' descriptors race
                # ahead of the chunk the PE needs next (ring round-robin
                # is priority inversion). Batch 0 ramps finely so the
                # first matmul starts ~8.5us; later batches load as one
                # DMA (32 KiB packets, fewest ring slots).
                lchunks = [2, 2, 4, 8, 8, 8] if b == 0 else [16, 16]
                g = 0
                for cw in lchunks:
                    nc.sync.dma_start(
                        xb[:, g * c_len : (g + cw) * c_len],
                        x[b, :, g * c_len : (g + cw) * c_len],
                    )
                    g += cw
                # store per chunk so the store stream starts ~one chunk
                # after the load stream instead of one batch after: the
                # 16 DMA engines then serve both queues wall-to-wall.
                # The last batch tapers so the final store's
                # load->matmul->evict->store tail is short.
                last_b = b == bcount - 1
                if b == 0:
                    # small first chunks: the store stream starts ~11us
                    schunks = [2, 2, 4, 8, 8, 8]
                elif last_b:
                    # taper: the final load->matmul->evict->store chain
                    # after the last load packet stays short
                    schunks = [8, 8, 8, 4, 2, 2]
                else:
                    schunks = [8, 8, 8, 8]
                g = 0
                for cw in schunks:
                    oc = opool.tile([P, 8 * c_len], _MM_DT, tag="oc")
                    for ii in range(cw):
                        i = g + ii
                        ps = pspool.tile([P, c_len], _F32, tag="ps")
                        dmax = min(i, D)
                        for nd, d in enumerate(range(dmax, -1, -1)):
                            j = i - d
                            wsl = 2 * d + (1 if j == 0 else 0)
                            nc.tensor.matmul(
                                ps[:],
                                wt[:, wsl * P : (wsl + 1) * P],
                                xb[:, j * c_len : (j + 1) * c_len],
                                start=(nd == 0),
                                stop=(nd == dmax),
                            )
                        dst = oc[:, ii * c_len : (ii + 1) * c_len]
                        if i % 2 == 0:
                            nc.scalar.copy(dst, ps[:])
                        else:
                            nc.vector.tensor_copy(dst, ps[:])
                    # gpsimd queue: the only DMA-capable engine with no
                    # other work, so store issue never blocks evictions
                    # (scalar/vector) or loads (sync)
                    nc.gpsimd.dma_start(
                        y[b, :, g * c_len : (g + cw) * c_len],
                        oc[:, : cw * c_len],
                    )
                    g += cw
    nc.compile()
    return nc


_MODULE_CACHE: dict = {}


def _get_module(D, **kw):
    key = (D, tuple(sorted(kw.items())))
    if key not in _MODULE_CACHE:
        _MODULE_CACHE[key] = _build(D, **kw)
    return _MODULE_CACHE[key]


def make_in_maps(x, alpha, beta, bcount=BC, n_cores=N_CORES):
    a = float(np.asarray(alpha).reshape(-1)[0])
    bt = float(np.asarray(beta).reshape(-1)[0])
    D, wts = _host_weights(a, bt)
    nw = 2 * (D + 1)
    # [nw, k, t] -> [k, nw*t] so the weight DMA is contiguous per partition
    wts16 = np.ascontiguousarray(
        wts.transpose(1, 0, 2).reshape(P, nw * P), dtype=_NP_DT
    )
    # [b, t, c] -> [b, t_in_block, blk, c]: partition t's whole batch is
    # one contiguous NBLK*C*2 = 32 KiB HBM chunk per DMA packet
    xt = x.reshape(B, NBLK, P, C).transpose(0, 2, 1, 3)
    in_maps = []
    for i in range(n_cores):
        xs = np.ascontiguousarray(
            xt[i * bcount : (i + 1) * bcount], dtype=_NP_DT
        ).reshape(bcount, P, NBLK * C)
        in_maps.append({"x": xs, "wts": wts16})
    return D, in_maps


def _run(x, alpha, beta, trace=False, **kw):
    x = np.asarray(x, dtype=np.float32)
    assert x.shape == (B, T, C), x.shape
    D, in_maps = make_in_maps(x, alpha, beta)
    nc = _get_module(D)
    res = run_bass_kernel_spmd(nc, in_maps, list(range(N_CORES)), trace=trace, **kw)
    yt = np.concatenate([res.results[i]["y"] for i in range(N_CORES)], axis=0)
    # invert the batch transpose: [b, t_in_block, blk, c] -> [b, t, c]
    out = (
        yt.reshape(B, P, NBLK, C)
        .transpose(0, 2, 1, 3)
        .reshape(B, T, C)
        .astype(np.float32)
    )
    return out, res


def kernel(x, alpha, beta):
    return _run(x, alpha, beta)[0]


# revision 42
# speedup vs baseline: 1.1214x; 1.0402x over previous
"""DEMA (double exponential smoothing) Trainium2 Bass kernel.

Math
----
Reference recurrence (per batch b, channel c, over time t):
    s0 = x[0], b0 = x[1] - x[0]
    s_t = a*x_t + (1-a)*(s_{t-1} + b_{t-1})
    b_t = bt*(s_t - s_{t-1}) + (1-bt)*b_{t-1}
    out = [s0, s_1, ..., s_{T-1}]

Eliminating the trend state gives a linear constant-coefficient 2nd-order
recurrence (exact; s_0 = x_0, s_1 = x_1):
    s_t = tau*s_{t-1} - delta*s_{t-2} + b0*x_t + b1*x_{t-1},  t >= 2
    tau = 2 - a - a*bt, delta = 1 - a, b0 = a, b1 = a*((1-a)*(1+bt) - tau)

So out = M @ x along time, where M is lower-triangular with Toeplitz body
M[t,k] = w_{t-k} (w = impulse response, w_j = tau*w_{j-1} - delta*w_{j-2})
plus two special leading columns for the x_0/x_1 initial conditions. The
poles satisfy |lambda| <= sqrt(1-a) < 1, so w decays geometrically and M
is effectively banded: blocking time into 128-chunks, out-block i only
needs input blocks j >= i-D, where D is chosen on host so the dropped
tail is below 1e-8 relative (D=1 for the graded PRNG alpha/beta).

The kernel is a causal blocked convolution on the TensorEngine:
    out_blk[i] = sum_{d=0..min(i,D)} W_d^T @ x_blk[i-d]       (PSUM accum)
with 128x128 weight blocks W_d (plus special j=0 variants carrying the
initial-condition columns) computed on host in float64 from the runtime
alpha/beta and shipped as a small input tensor.

Performance notes (236.6us fp32 ancestor -> 96.4-110us measured;
run-to-run spread is HBM/DMA service-rate noise across runs — judge
changes by trace structure, not single wall-clock samples):
- Everything moves in float16 (x, weights, y; PSUM accumulates fp32).
  The graded accuracy gate is 2e-2 relative; fp16 quantization costs
  ~3.6e-4. PE time drops 4x (fp16 is 1 pass/column vs 4 for fp32) and
  HBM traffic halves to ~33.7 MB/core.
- Host ships x batch-transposed as [128, NBLK*C]: partition t holds
  row t of every 128-block back to back, so a whole-batch DMA is 128
  packets of NBLK*C*2 = 32 KiB contiguous HBM each. The fp32 ancestor
  moved one 2 KiB row per packet. The 16 DMA engines cap at ~27 GB/s
  each (~430 GB/s/core aggregate, reached for packets >= 8 KiB), so
  the 33.7 MB floor is ~80us of saturated DMA: this kernel is
  DMA-bound, with loads and stores each needing ~215 GB/s sustained.
  y uses the same layout; the host inverse transpose/cast is not on
  the HW critical path.
- Queue topology (measured, counterintuitive): ALL loads on the sync
  ring in exact consumption order (a second load ring lets later
  batches' descriptors race ahead of the chunk the PE needs next —
  priority inversion), stores on the gpsimd ring (the only
  DMA-capable engine with no other work: store issues on
  scalar/vector stall evictions because engines execute in order),
  PSUM evictions alternate ScalarE/VectorE. Loads go as half-batch
  (16-block) sub-DMAs: 16 KiB packets hit the same ~27 GB/s/engine as
  32 KiB, and halving the batch-boundary gate cut total PE stall from
  ~21us to ~13us (the PE outruns its ~50% share of the load stream,
  so it waits at every batch start). Stores go out in 8-block chunks
  so the store stream starts ~14us in and the DMA engines serve both
  directions wall-to-wall.
- Batch 0 ramps with small load/store chunks (first matmul at ~11us)
  and the last batch tapers its stores (8,8,8,4,2,2) so the final
  load->matmul->evict->store chain stays short.

Sharding: batch 32 -> 4 per core across 8 cores (data parallel; the
recurrence is independent per (b, c)).
"""

import numpy as np

import concourse.bacc as bacc
import concourse.bass as bass
import concourse.mybir as mybir
from concourse import tile
from concourse.bass_utils import run_bass_kernel_spmd

N_CORES = 8
P = 128            # SBUF partitions == time-block length
B, T, C = 32, 4096, 512
BC = B // N_CORES  # batches per core
NBLK = T // P      # 32 time blocks
CHUNK = 8          # blocks per fill/drain chunk (8*512*2B = 8 KiB/partition)

_F32 = mybir.dt.float32
_MM_DT = mybir.dt.float16
_NP_DT = np.float16


def _host_weights(a: float, bt: float, tol: float = 1e-8):
    """Impulse response + IC columns -> (D, wts[2*(D+1), 128, 128]) lhsT-layout."""
    tau = 2.0 - a - a * bt
    delta = 1.0 - a
    b0 = a
    b1 = a * ((1.0 - a) * (1.0 + bt) - tau)
    n = T
    w = np.zeros(n)
    c0 = np.zeros(n)
    c1 = np.zeros(n)
    w[0] = b0
    w[1] = tau * b0 + b1
    c0[0] = 1.0
    c1[1] = 1.0
    for j in range(2, n):
        w[j] = tau * w[j - 1] - delta * w[j - 2]
        c0[j] = tau * c0[j - 1] - delta * c0[j - 2]
        c1[j] = tau * c1[j - 1] - delta * c1[j - 2] + (b1 if j == 2 else 0.0)
    wnorm = max(np.sqrt((w ** 2).sum()), 1.0)
    D = NBLK - 1
    for d in range(NBLK):
        tail = np.sqrt(
            (w[P * d + 1 :] ** 2).sum()
            + (c0[P * (d + 1) :] ** 2).sum()
            + (c1[P * (d + 1) :] ** 2).sum()
        )
        if tail <= tol * wnorm:
            D = d
            break
    # lhsT layout [k, t]: out[t, n] = sum_k W[k, t] * x[k, n]
    wts = np.zeros((2 * (D + 1), P, P), np.float32)
    kk = np.arange(P)[:, None]
    tt = np.arange(P)[None, :]
    for d in range(D + 1):
        lag = P * d + tt - kk          # [k, t] lag matrix
        Tm = np.where((lag >= 0) & (lag < n), w[np.clip(lag, 0, n - 1)], 0.0)
        Sm = Tm.copy()
        Sm[0, :] = c0[P * d : P * d + P]
        Sm[1, :] = c1[P * d : P * d + P]
        wts[2 * d] = Tm
        wts[2 * d + 1] = Sm
    return D, wts


def _build(D, bcount=BC, t_len=T, c_len=C):
    """Build + compile the per-core SPMD module for diagonal depth D."""
    nblk = t_len // P
    nw = 2 * (D + 1)
    nc = bacc.Bacc("TRN2", target_bir_lowering=False, debug=False)
    # x/y live in HBM batch-transposed: [b, t_in_block, blk*C + c]
    x = nc.dram_tensor("x", [bcount, P, nblk * c_len], _MM_DT, kind="ExternalInput")
    wd = nc.dram_tensor("wts", [P, nw * P], _MM_DT, kind="ExternalInput")
    y = nc.dram_tensor("y", [bcount, P, nblk * c_len], _MM_DT, kind="ExternalOutput")

    with tile.TileContext(nc) as tc:
        with (
            tc.tile_pool(name="wpool", bufs=1) as wpool,
            tc.tile_pool(name="xpool", bufs=3) as xpool,
            tc.tile_pool(name="psum", bufs=8, space="PSUM") as pspool,
            tc.tile_pool(name="opool", bufs=4) as opool,
        ):
            wt = wpool.tile([P, nw * P], _MM_DT)
            nc.sync.dma_start(wt[:], wd[:])

            for b in range(bcount):
                xb = xpool.tile([P, nblk * c_len], _MM_DT, tag="xb")
                # chunked loads, alternating between the sync and gpsimd
                # DMA queues (both issuing engines are otherwise idle, so
                # issue is never delayed): with stores on a third
                # (scalar) queue, the per-ring round-robin of the 16 DMA
                # engines gives the load stream a 2/3 share, so loads
                # (which gate the PE, which gates evictions/stores)
                # finish early and stores soak up the engines afterwards.
                # Batch 0 ramps with small chunks so the first matmul
                # starts ASAP.
                # all loads on ONE ring (sync) in consumption order: a
                # second load ring lets later batches' descriptors race
                # ahead of the chunk the PE needs next (ring round-robin
                # is priority inversion). Batch 0 ramps finely so the
                # first matmul starts ~8.5us; later batches load as one
                # DMA (32 KiB packets, fewest ring slots).
                lchunks = [2, 2, 4, 8, 8, 8] if b == 0 else [16, 16]
                g = 0
                for cw in lchunks:
                    nc.sync.dma_start(
                        xb[:, g * c_len : (g + cw) * c_len],
                        x[b, :, g * c_len : (g + cw) * c_len],
                    )
                    g += cw
                # store per chunk so the store stream starts ~one chunk
                # after the load stream instead of one batch after: the
                # 16 DMA engines then serve both queues wall-to-wall.
                # The last batch tapers so the final store's
                # load->matmul->evict->store tail is short.
                last_b = b == bcount - 1
                if b == 0:
                    # small first chunks: the store stream starts ~11us
                    schunks = [2, 2, 4, 8, 8, 8]
                elif last_b:
                    # taper: the final load->matmul->evict->store chain
                    # after the last load packet stays short
                    schunks = [8, 8, 8, 4, 2, 2]
                else:
                    schunks = [8, 8, 8, 8]
                g = 0
                for cw in schunks:
                    oc = opool.tile([P, 8 * c_len], _MM_DT, tag="oc")
                    for ii in range(cw):
                        i = g + ii
                        ps = pspool.tile([P, c_len], _F32, tag="ps")
                        dmax = min(i, D)
                        for nd, d in enumerate(range(dmax, -1, -1)):
                            j = i - d
                            wsl = 2 * d + (1 if j == 0 else 0)
                            nc.tensor.matmul(
                                ps[:],
                                wt[:, wsl * P : (wsl + 1) * P],
                                xb[:, j * c_len : (j + 1) * c_len],
                                start=(nd == 0),
                                stop=(nd == dmax),
                            )
                        dst = oc[:, ii * c_len : (ii + 1) * c_len]
                        if i % 2 == 0:
                            nc.scalar.copy(dst, ps[:])
                        else:
                            nc.vector.tensor_copy(dst, ps[:])
                    # gpsimd queue: the only DMA-capable engine with no
                    # other work, so store issue never blocks evictions
                    # (scalar/vector) or loads (sync)
                    nc.gpsimd.dma_start(
                        y[b, :, g * c_len : (g + cw) * c_len],
                        oc[:, : cw * c_len],
                    )
                    g += cw
    nc.compile()
    return nc


_MODULE_CACHE: dict = {}


def _get_module(D, **kw):
    key = (D, tuple(sorted(kw.items())))
    if key not in _MODULE_CACHE:
        _MODULE_CACHE[key] = _build(D, **kw)
    return _MODULE_CACHE[key]


def make_in_maps(x, alpha, beta, bcount=BC, n_cores=N_CORES):
    a = float(np.asarray(alpha).reshape(-1)[0])
    bt = float(np.asarray(beta).reshape(-1)[0])
    D, wts = _host_weights(a, bt)
    nw = 2 * (D + 1)
    # [nw, k, t] -> [k, nw*t] so the weight DMA is contiguous per partition
    wts16 = np.ascontiguousarray(
        wts.transpose(1, 0, 2).reshape(P, nw * P), dtype=_NP_DT
    )
    # [b, t, c] -> [b, t_in_block, blk, c]: partition t's whole batch is
    # one contiguous NBLK*C*2 = 32 KiB HBM chunk per DMA packet
    xt = x.reshape(B, NBLK, P, C).transpose(0, 2, 1, 3)
    in_maps = []
    for i in range(n_cores):
        xs = np.ascontiguousarray(
            xt[i * bcount : (i + 1) * bcount], dtype=_NP_DT
        ).reshape(bcount, P, NBLK * C)
        in_maps.append({"x": xs, "wts": wts16})
    return D, in_maps


def _run(x, alpha, beta, trace=False, **kw):
    x = np.asarray(x, dtype=np.float32)
    assert x.shape == (B, T, C), x.shape
    D, in_maps = make_in_maps(x, alpha, beta)
    nc = _get_module(D)
    res = run_bass_kernel_spmd(nc, in_maps, list(range(N_CORES)), trace=trace, **kw)
    yt = np.concatenate([res.results[i]["y"] for i in range(N_CORES)], axis=0)
    # invert the batch transpose: [b, t_in_block, blk, c] -> [b, t, c]
    out = (
        yt.reshape(B, P, NBLK, C)
        .transpose(0, 2, 1, 3)
        .reshape(B, T, C)
        .astype(np.float32)
    )
    return out, res


def kernel(x, alpha, beta):
    return _run(x, alpha, beta)[0]
